# revision 1
# baseline (speedup 1.0000x reference)
import sys

if "/opt/trn_rl_repo" not in sys.path:
    sys.path.insert(0, "/opt/trn_rl_repo")

import numpy as np

# Problem: y = LeakyReLU((conv2d(x, w, VALID) + bias) / 2, slope=0.01)
#   x: (32, 128, 130, 130) f32, w: (256, 128, 3, 3) f32, b: (256,) f32
#   y: (32, 256, 128, 128) f32
# Sharding: data-parallel over batch, 4 images per core on 8 cores.
# Per core: conv as implicit GEMM in fp16 (1 cycle/row on the PE; fp32
# accumulation in PSUM; ~2.4e-4 rel err, vs 1.2e-4 for fp32r at +8% time) —
# for each output tile of 4 rows x 128 cols, accumulate 9 matmuls
# (one per 3x3 tap) of [K=128(C_in), M=128(C_out)] x [K=128, N=512] into one
# PSUM bank, then a single fused ACT epilogue
# Prelu(psum*0.5 + 0.5*bias, alpha=0.01) straight out of PSUM.
# x streams in row-chunks per image (the first chunk is small so the PE
# starts early); weights load as two j-halves right after the first x chunk,
# all on the sync (HWDGE) queue whose transfers run on parallel HW queues.

N_CORES = 8
IMGS_PER_CORE = 4
C_IN = 128
C_OUT = 256
H_IN = 130
W_IN = 130
H_OUT = 128
W_OUT = 128
ROWS_PER_TILE = 4            # output rows per matmul tile -> N = 4*128 = 512
N_TILE = ROWS_PER_TILE * W_OUT
DIVISOR = 2.0
SLOPE = 0.01

# (start_out_row, n_out_rows) chunk schedules; image 0 front-loads a small
# chunk so the first matmuls start as early as possible.
FIRST_IMG_CHUNKS = [(0, 8), (8, 40), (48, 40), (88, 40)]
OTHER_IMG_CHUNKS = [(0, 32), (32, 32), (64, 32), (96, 32)]
MAX_CHUNK_IN_ROWS = max(r for _, r in FIRST_IMG_CHUNKS + OTHER_IMG_CHUNKS) + 2
# SBUF x row stride. (Aligned-stride/offset variants and on-chip shifted
# copies were measured: any per-MM AP-alignment gain is eaten by the extra
# SBUF traffic; plain rows are fastest overall.)
ROW_STRIDE = W_IN

_CACHE = {}


def _build():
    import concourse.tile as tile
    import concourse.mybir as mybir
    from concourse import bacc

    F32 = mybir.dt.float32
    F16 = mybir.dt.float16

    nc = bacc.Bacc(
        "TRN2",
        target_bir_lowering=False,
        debug=False,
        enable_asserts=True,
        num_devices=N_CORES,
    )

    x_d = nc.dram_tensor(
        "x", [IMGS_PER_CORE * C_IN, H_IN * W_IN], F16, kind="ExternalInput"
    ).ap()
    # w free layout: j*1152 + ki*128 + co_lo   (j = cout tile, ki = 3x3 tap)
    w_d = nc.dram_tensor("w", [C_IN, 9 * C_OUT], F16, kind="ExternalInput").ap()
    b_d = nc.dram_tensor("b", [C_OUT // 2, 2], F32, kind="ExternalInput").ap()
    y_d = nc.dram_tensor(
        "y", [IMGS_PER_CORE * C_OUT, H_OUT * W_OUT], F32, kind="ExternalOutput"
    ).ap()

    with tile.TileContext(nc) as tc:
        with (
            tc.tile_pool(name="const", bufs=1) as const_pool,
            tc.tile_pool(name="xbuf", bufs=6) as x_pool,
            tc.tile_pool(name="psum", bufs=8, space="PSUM") as psum_pool,
            tc.tile_pool(name="obuf", bufs=8) as out_pool,
        ):
            w_sb = const_pool.tile([C_IN, 9 * C_OUT], F16)
            b_sb = const_pool.tile([C_OUT // 2, 2], F32)
            consts_loaded = False

            for n in range(IMGS_PER_CORE):
                chunks = FIRST_IMG_CHUNKS if n == 0 else OTHER_IMG_CHUNKS
                for row0, nrows in chunks:
                    in_rows = nrows + 2
                    xc = x_pool.tile([C_IN, MAX_CHUNK_IN_ROWS * ROW_STRIDE], F16)
                    xv = xc[:, : in_rows * ROW_STRIDE].rearrange(
                        "p (h w) -> p h w", h=in_rows
                    )
                    nc.sync.dma_start(
                        xv[:, :, 0:W_IN],
                        x_d[
                            n * C_IN : (n + 1) * C_IN,
                            row0 * W_IN : (row0 + in_rows) * W_IN,
                        ].rearrange("p (h w) -> p h w", h=in_rows),
                    )
                    if not consts_loaded:
                        # issue right after the first (small) x chunk so the
                        # HW DMA queues run them all in parallel
                        consts_loaded = True
                        for j in range(2):
                            nc.sync.dma_start(
                                w_sb[:, j * 1152 : (j + 1) * 1152],
                                w_d[:, j * 1152 : (j + 1) * 1152],
                            )
                        nc.sync.dma_start(b_sb[:], b_d[:])
                    for gl in range(nrows // ROWS_PER_TILE):
                        g = row0 // ROWS_PER_TILE + gl
                        for j in range(2):  # cout tile
                            ps = psum_pool.tile([128, N_TILE], F32)
                            for ki in range(9):
                                kh, kw = divmod(ki, 3)
                                r0 = gl * ROWS_PER_TILE + kh
                                rhs = xv[:, r0 : r0 + ROWS_PER_TILE, kw : kw + W_OUT]
                                nc.tensor.matmul(
                                    ps[:],
                                    w_sb[
                                        :,
                                        j * 1152 + ki * 128 : j * 1152 + ki * 128 + 128,
                                    ],
                                    rhs,
                                    start=(ki == 0),
                                    stop=(ki == 8),
                                )
                            ot = out_pool.tile([128, N_TILE], F32)
                            nc.scalar.activation(
                                ot[:],
                                ps[:],
                                mybir.ActivationFunctionType.Prelu,
                                bias=b_sb[:, j : j + 1],
                                scale=1.0 / DIVISOR,
                                alpha=SLOPE,
                            )
                            nc.sync.dma_start(
                                y_d[
                                    n * C_OUT + j * 128 : n * C_OUT + (j + 1) * 128,
                                    g * N_TILE : (g + 1) * N_TILE,
                                ],
                                ot[:],
                            )

    nc.compile()
    return nc


# Results of the last hardware run (for test.py to pull profiling info from).
LAST_RESULT = None


def kernel(x, weight, bias):
    from concourse.bass_utils import run_bass_kernel_spmd

    global LAST_RESULT

    if "nc" not in _CACHE:
        _CACHE["nc"] = _build()
    nc = _CACHE["nc"]

    x = np.ascontiguousarray(x, dtype=np.float32).astype(np.float16)
    # [co, ci, kh, kw] -> [ci, j, ki, co_lo] -> [128, 2304]
    wt = np.ascontiguousarray(
        weight.astype(np.float32)
        .transpose(1, 2, 3, 0)
        .reshape(C_IN, 9, 2, 128)
        .transpose(0, 2, 1, 3)
    ).reshape(C_IN, 9 * C_OUT).astype(np.float16)
    # bias*0.5 as [128, 2]: column j = cout tile j
    bh = np.ascontiguousarray(
        (bias.astype(np.float32) / DIVISOR).reshape(2, 128).T
    )

    in_maps = []
    for c in range(N_CORES):
        xs = x[c * IMGS_PER_CORE : (c + 1) * IMGS_PER_CORE].reshape(
            IMGS_PER_CORE * C_IN, H_IN * W_IN
        )
        in_maps.append({"x": xs, "w": wt, "b": bh})

    res = run_bass_kernel_spmd(nc, in_maps, core_ids=list(range(N_CORES)))
    LAST_RESULT = res
    out = np.concatenate(
        [
            r["y"].reshape(IMGS_PER_CORE, C_OUT, H_OUT, W_OUT)
            for r in res.results
        ],
        axis=0,
    )
    return out



# revision 5
# speedup vs baseline: 1.0917x; 1.0917x over previous
import sys

if "/opt/trn_rl_repo" not in sys.path:
    sys.path.insert(0, "/opt/trn_rl_repo")

import numpy as np

# Problem: y = LeakyReLU((conv2d(x, w, VALID) + bias) / 2, slope=0.01)
#   x: (32, 128, 130, 130) f32, w: (256, 128, 3, 3) f32, b: (256,) f32
#   y: (32, 256, 128, 128) f32
# Sharding: data-parallel over batch, 4 images per core on 8 cores.
#
# Per core: 1D Winograd F(2,3) along the width, direct accumulation over the
# 3 vertical taps. For each pair of output columns (2q, 2q+1) the 3
# horizontal taps become 4 multiply-terms on transformed inputs:
#   m0 = (d0-d2)*g0, m1 = (d1+d2)*(g0+g1+g2)/2, m2 = (d2-d1)*(g0-g1+g2)/2,
#   m3 = (d1-d3)*g2;  y0 = m0+m1+m2, y1 = m1-m2-m3
# so the GEMM does 12 matmuls (4 m-terms x 3 kh) per 8-row x 128-col output
# block per cout half instead of 18 -> 1.5x fewer PE cycles. The input
# transform (4 fp16 tensor_tensor ops per chunk) runs on DVE; the m-planes
# are evacuated PSUM->SBUF fp16 by one ACT copy (with the /2 folded in); DVE
# does the 4 fp16 output-transform adds; ACT applies Prelu(y + b/2). Output
# rows are written as [y0(64) || y1(64)] in fp16; the host interleaves the
# column pairs and casts to fp32.
#
# Host-side layout prep: x is sent fp16 with each row's columns split into
# even (65) then odd (65) halves so all DVE transform slices are dense.

N_CORES = 8
IMGS_PER_CORE = 4
C_IN = 128
C_OUT = 256
H_IN = 130
W_IN = 130
H_OUT = 128
W_OUT = 128
NQ = W_OUT // 2              # 64 column pairs
ROWS_PER_BLOCK = 8           # output rows per GEMM block -> N = 8*64 = 512
N_TILE = ROWS_PER_BLOCK * NQ
DIVISOR = 2.0
SLOPE = 0.01

# (start_out_row, n_out_rows) chunk schedules; image 0 front-loads a small
# chunk so the first matmuls start as early as possible.
FIRST_IMG_CHUNKS = [(0, 8), (8, 40), (48, 40), (88, 40)]
OTHER_IMG_CHUNKS = [(0, 32), (32, 32), (64, 32), (96, 32)]
MAX_CHUNK_IN_ROWS = max(r for _, r in FIRST_IMG_CHUNKS + OTHER_IMG_CHUNKS) + 2

_CACHE = {}


def _build():
    import concourse.tile as tile
    import concourse.mybir as mybir
    from concourse import bacc

    F32 = mybir.dt.float32
    F16 = mybir.dt.float16

    nc = bacc.Bacc(
        "TRN2",
        target_bir_lowering=False,
        debug=False,
        enable_asserts=True,
        num_devices=N_CORES,
    )

    # x row layout: [even cols 0,2,..,128 (65) || odd cols 1,3,..,129 (65)]
    x_d = nc.dram_tensor(
        "x", [IMGS_PER_CORE * C_IN, H_IN * W_IN], F16, kind="ExternalInput"
    ).ap()
    # w' free layout: (kh*4 + m)*256 + j*128 + co_lo
    w_d = nc.dram_tensor("w", [C_IN, 12 * C_OUT], F16, kind="ExternalInput").ap()
    b_d = nc.dram_tensor("b", [C_OUT // 2, 2], F32, kind="ExternalInput").ap()
    # y row layout: [y0 half (64) || y1 half (64)] fp16
    y_d = nc.dram_tensor(
        "y", [IMGS_PER_CORE * C_OUT, H_OUT * W_OUT], F16, kind="ExternalOutput"
    ).ap()

    with tile.TileContext(nc) as tc:
        with (
            tc.tile_pool(name="const", bufs=1) as const_pool,
            tc.tile_pool(name="xbuf", bufs=4) as x_pool,
            tc.tile_pool(name="mbuf", bufs=3) as m_pool,
            tc.tile_pool(name="psum", bufs=2, space="PSUM") as psum_pool,
            tc.tile_pool(name="msb", bufs=4) as msb_pool,
            tc.tile_pool(name="ybuf", bufs=6) as y_pool,
        ):
            w_sb = const_pool.tile([C_IN, 12 * C_OUT], F16)
            b_sb = const_pool.tile([C_OUT // 2, 2], F32)
            consts_loaded = False

            for n in range(IMGS_PER_CORE):
                chunks = FIRST_IMG_CHUNKS if n == 0 else OTHER_IMG_CHUNKS
                for row0, nrows in chunks:
                    in_rows = nrows + 2
                    xc = x_pool.tile([C_IN, MAX_CHUNK_IN_ROWS * W_IN], F16)
                    xv = xc[:, : in_rows * W_IN].rearrange(
                        "p (h w) -> p h w", h=in_rows
                    )
                    nc.sync.dma_start(
                        xv[:, :, :],
                        x_d[
                            n * C_IN : (n + 1) * C_IN,
                            row0 * W_IN : (row0 + in_rows) * W_IN,
                        ].rearrange("p (h w) -> p h w", h=in_rows),
                    )
                    if not consts_loaded:
                        consts_loaded = True
                        for j in range(2):
                            nc.sync.dma_start(
                                w_sb[:, j * 1536 : (j + 1) * 1536],
                                w_d[:, j * 1536 : (j + 1) * 1536],
                            )
                        nc.sync.dma_start(b_sb[:], b_d[:])

                    # input transform on DVE: 4 fp16 tensor_tensor ops
                    mb = m_pool.tile([C_IN, MAX_CHUNK_IN_ROWS, 4, NQ], F16)
                    d0 = xv[:, :, 0:NQ]
                    d1 = xv[:, :, 65 : 65 + NQ]
                    d2 = xv[:, :, 1 : 1 + NQ]
                    d3 = xv[:, :, 66 : 66 + NQ]
                    mv = mb[:, :in_rows]
                    nc.vector.tensor_sub(mv[:, :, 0], d0, d2)
                    nc.vector.tensor_add(mv[:, :, 1], d1, d2)
                    nc.vector.tensor_sub(mv[:, :, 2], d2, d1)
                    nc.vector.tensor_sub(mv[:, :, 3], d1, d3)

                    for bl in range(nrows // ROWS_PER_BLOCK):
                        r0 = bl * ROWS_PER_BLOCK
                        g = (row0 + r0) // ROWS_PER_BLOCK
                        for j in range(2):
                            ps = psum_pool.tile([128, 4 * N_TILE], F32)
                            for m in range(4):
                                for kh in range(3):
                                    woff = ((kh * 4 + m) * 2 + j) * 128
                                    nc.tensor.matmul(
                                        ps[:, m * N_TILE : (m + 1) * N_TILE],
                                        w_sb[:, woff : woff + 128],
                                        mv[:, r0 + kh : r0 + kh + ROWS_PER_BLOCK, m],
                                        start=(kh == 0),
                                        stop=(kh == 2),
                                    )
                            # evacuate all 4 m-planes, with the /2 folded in
                            ms = msb_pool.tile([128, 4 * N_TILE], F16)
                            nc.scalar.activation(
                                ms[:],
                                ps[:],
                                mybir.ActivationFunctionType.Copy,
                                bias=0.0,
                                scale=1.0 / DIVISOR,
                            )
                            m0 = ms[:, 0 * N_TILE : 1 * N_TILE]
                            m1 = ms[:, 1 * N_TILE : 2 * N_TILE]
                            m2 = ms[:, 2 * N_TILE : 3 * N_TILE]
                            m3 = ms[:, 3 * N_TILE : 4 * N_TILE]
                            yt = y_pool.tile([128, 2 * N_TILE], F16)
                            y0 = yt[:, :N_TILE]
                            y1 = yt[:, N_TILE:]
                            # y0 = (m0+m1)+m2 ; y1 = (m1-m2)-m3 on DVE
                            nc.vector.tensor_add(y0, m0, m1)
                            nc.vector.tensor_add(y0, y0, m2)
                            nc.vector.tensor_sub(y1, m1, m2)
                            nc.vector.tensor_sub(y1, y1, m3)
                            # fused epilogue: Prelu(y + b/2)
                            yo = y_pool.tile([128, 2 * N_TILE], F16)
                            nc.scalar.activation(
                                yo[:],
                                yt[:],
                                mybir.ActivationFunctionType.Prelu,
                                bias=b_sb[:, j : j + 1],
                                scale=1.0,
                                alpha=SLOPE,
                            )
                            # dense [i(2), h(8), q(64)] block; host un-interleaves
                            nc.sync.dma_start(
                                y_d[
                                    n * C_OUT + j * 128 : n * C_OUT + (j + 1) * 128,
                                    g * N_TILE * 2 : (g + 1) * N_TILE * 2,
                                ],
                                yo[:],
                            )

    nc.compile()
    return nc


# Results of the last hardware run (for test.py to pull profiling info from).
LAST_RESULT = None


def kernel(x, weight, bias):
    from concourse.bass_utils import run_bass_kernel_spmd

    global LAST_RESULT

    if "nc" not in _CACHE:
        _CACHE["nc"] = _build()
    nc = _CACHE["nc"]

    x = np.ascontiguousarray(x, dtype=np.float32).astype(np.float16)
    # split each row's columns into even(65) || odd(65)
    xs = x.reshape(32, C_IN, H_IN, 65, 2)
    x_eo = np.concatenate([xs[..., 0], xs[..., 1]], axis=3)  # [32,128,130,130]

    # weight transform along kw: w'[kh, m] for m in 0..3
    wf = weight.astype(np.float32)  # [co, ci, kh, kw]
    g0, g1, g2 = wf[..., 0], wf[..., 1], wf[..., 2]
    wm = np.stack(
        [g0, (g0 + g1 + g2) * 0.5, (g0 - g1 + g2) * 0.5, g2], axis=3
    )  # [co, ci, kh, m]
    # -> [ci, kh, m, j, co_lo] -> [128, 12*256]
    wt = np.ascontiguousarray(
        wm.reshape(2, 128, C_IN, 3, 4).transpose(2, 3, 4, 0, 1)
    ).reshape(C_IN, 12 * C_OUT).astype(np.float16)
    # bias/2 as [128, 2]: column j = cout half j
    bh = np.ascontiguousarray(
        (bias.astype(np.float32) / DIVISOR).reshape(2, 128).T
    )

    in_maps = []
    for c in range(N_CORES):
        xc = x_eo[c * IMGS_PER_CORE : (c + 1) * IMGS_PER_CORE].reshape(
            IMGS_PER_CORE * C_IN, H_IN * W_IN
        )
        in_maps.append({"x": xc, "w": wt, "b": bh})

    res = run_bass_kernel_spmd(nc, in_maps, core_ids=list(range(N_CORES)))
    LAST_RESULT = res
    # each 8-row block is stored dense as [i(2), h(8), q(64)]; un-interleave
    y = np.stack([r["y"] for r in res.results]).reshape(
        32, C_OUT, H_OUT // 8, 2, 8, 64
    )
    out = np.ascontiguousarray(y.transpose(0, 1, 2, 4, 5, 3)).reshape(
        32, C_OUT, H_OUT, W_OUT
    ).astype(np.float32)
    return out
